# revision 21
# baseline (speedup 1.0000x reference)
"""Trainium2 Bass kernel for nn_Denoiser_73598559584966.

Full-sequence self-attention (Q=K=V, no scaling) over x: [4, 16, 16, 16, 64]
  t = x.reshape(B, 4096, 64); out = softmax(t @ t^T) @ t

Sharding: 8 cores = 4 batches x 2 query-halves. Each core: 2048 queries
vs the full 4096 keys/values of its batch. No collectives.

v3: two passes over chunk-pairs, pair-inner, streamed as one pipeline.
Pass 0 covers query chunks 0,1; pass 1 covers chunks 2,3. Per pass the
output accumulator o_acc is only [128, 1024] f32 (2 PSUM banks), which
frees SIX banks for THREE [128, 1024] score buffers -- enough slack
that the exp engines never gate the PE's score-bank reuse (v2's 2-buffer
ping-pong made every half-period wait on a full exp op + 2 sem hops and
ran 2x slow).

Period (one pair p, 6 N=512 streams, ~1280ns):
  [ s(p,c0) | pvA(p-2,c0) pvA(p-2,c1) | s(p,c1) | pvB(p-2,c0) pvB(p-2,c1) ]
- PV trails the scores by TWO pairs so no PV ever waits on an exp op.
- Weight sequence K(p), vA(p-2), K(p) again (still resident -- the two
  score slots bracket only one other weight so K survives in its array
  slot), vB(p-2): 3 weight loads per 6 streams, each with >=1 full
  stream of prefetch lead (v1 paid ~210ns/3-streams of exposed loads).
- exp granularity is one whole chunk per op: ScalarE does exact exp on
  chunk c0 [128, 1024] (~1049ns), VectorE does Schraudolph bf16-bitcast
  exp on chunk c1 (~1263ns). Staggered score slots give each score
  buffer a ~1707ns reuse window vs ~1550/2400ns read-done chains.

K layout is packed even/odd: khp[0:64, p*128:+128] = K^T of tile 2p,
khp[64:128, same cols] = K^T of tile 2p+1 -> the pair's two concurrent
row-group score matmuls share one 128-col weight block (and K DMA
halves vs duplicating K^T). Softmax shift is global per core
(shift = max|q|*max|k| - 30); it cancels exactly in the ratio and
keeps relevant s-shift in bf16-normal / int16-safe range.

o_acc rows: 0-63 = V out, row 64 = denominator (ones column in the V
weights), 65+ = garbage from the 65-col-stride V weight blocks (never
read). Host divides and transposes while gathering shards.
"""
import numpy as np

B_, D_, H_, W_, C_ = 4, 16, 16, 16, 64
NTOK = D_ * H_ * W_          # 4096 tokens per batch
NQ = NTOK // 2               # 2048 queries per core
NCORES = 8
NKT = NTOK // 128            # 32 key tiles
NPAIR = NKT // 2             # 16 packed key-tile pairs
NPASS = 2                    # chunk-pair passes (chunks 2t, 2t+1)
CHW = 512                    # queries per chunk
NG = 4                       # DMA groups over key-tile pairs
GPR = NPAIR // NG            # 4 pairs per group

AEXP = 184.66350558899108    # 128 / ln 2  (bf16 Schraudolph scale)
C_SCH = 5.590103149414062    # Schraudolph bias-correction (bf16 grid)
MARGIN = 30.0                # shift = core score bound - MARGIN
NWARM = 3                    # PE warmup matmuls under the DMA prefix
TRAIL = 2                    # pv trails scores by TRAIL pairs

_CACHE = {}


def _build_nc():
    import concourse.bacc as bacc
    import concourse.mybir as mybir
    from concourse.tile import TileContext

    f32 = mybir.dt.float32
    i16 = mybir.dt.int16
    bf16 = mybir.dt.bfloat16
    EXP = mybir.ActivationFunctionType.Exp
    ADD = mybir.AluOpType.add
    MAX = mybir.AluOpType.max
    nc = bacc.Bacc("TRN2", target_bir_lowering=False, debug=False)

    qhh = nc.dram_tensor("qhh", [128, NQ], bf16, kind="ExternalInput")
    khp = nc.dram_tensor("khp", [128, NPAIR * 128], bf16, kind="ExternalInput")
    vpk = nc.dram_tensor("vpk", [128, NKT * 65 + 64], bf16,
                         kind="ExternalInput")
    bsh = nc.dram_tensor("bsh", [128, 1], f32, kind="ExternalInput")
    bdv = nc.dram_tensor("bdv", [128, 1], f32, kind="ExternalInput")
    out = nc.dram_tensor("out", [65, NQ], f32, kind="ExternalOutput")

    KGW = GPR * 128           # khp cols per DMA group (512)
    VGW = GPR * 2 * 65        # vpk cols per DMA group (520)
    NPER = NPASS * NPAIR + TRAIL
    with TileContext(nc) as tc:
        with (
            tc.tile_pool(name="const", bufs=1) as const,
            tc.tile_pool(name="pp", bufs=3) as pp,
            tc.tile_pool(name="sbo", bufs=1) as sbo,
            tc.tile_pool(name="ps_s", bufs=3, space="PSUM") as ps_s,
            tc.tile_pool(name="ps_o", bufs=1, space="PSUM") as ps_o,
        ):
            # ---- PE + ACT warmup during the DMA prefix ----
            wz = const.tile([128, 512], bf16, tag="wz")
            nc.vector.memset(wz, 0.0)
            wexp = const.tile([128, 1], f32, tag="wexp")
            nc.scalar.activation(wexp, wz[:, 0:1], EXP)  # pull exp table load

            # ---- input DMAs: first-needed operands first ----
            qhh_t = const.tile([128, NQ], bf16, tag="qhh")
            khp_g = []
            vpk_g = []
            for g in range(NG):
                kt_ = const.tile([128, KGW], bf16, tag=f"khp_{g}")
                khp_g.append(kt_)
                kt_ = const.tile([128, VGW + 63], bf16, tag=f"vpk_{g}")
                vpk_g.append(kt_)
            nc.sync.dma_start(out=khp_g[0][:, 0:128], in_=khp[:, 0:128])
            nc.sync.dma_start(out=qhh_t[:, 0:1024], in_=qhh[:, 0:1024])
            bsh_t = const.tile([128, 1], f32, tag="bsh")
            nc.sync.dma_start(out=bsh_t, in_=bsh[:, :])
            bdv_t = const.tile([128, 1], f32, tag="bdv")
            nc.sync.dma_start(out=bdv_t, in_=bdv[:, :])
            nc.sync.dma_start(out=vpk_g[0], in_=vpk[:, 0:VGW + 63])
            nc.sync.dma_start(out=khp_g[0][:, 128:KGW], in_=khp[:, 128:KGW])
            for g in range(1, NG):
                nc.sync.dma_start(
                    out=khp_g[g], in_=khp[:, g * KGW:(g + 1) * KGW])
                nc.sync.dma_start(
                    out=vpk_g[g], in_=vpk[:, g * VGW:(g + 1) * VGW + 63])
            nc.sync.dma_start(out=qhh_t[:, 1024:2048], in_=qhh[:, 1024:2048])

            o_accs = [None, None]        # per pass
            p_hist = {}                  # (pass, pair) -> p_t
            o_sb = sbo.tile([65, NQ], f32, tag="osb")
            prev_s_c1 = None             # last period's c1 score buffer

            def emit_scores(pr, j, s_c, t):
                g = pr // GPR
                kc = (pr - g * GPR) * 128
                c = 2 * t + j
                qs = slice(c * CHW, (c + 1) * CHW)
                nc.tensor.matmul(
                    s_c[:, 0:512],
                    khp_g[g][0:64, kc:kc + 128], qhh_t[0:64, qs],
                    start=True, stop=True, skip_group_check=True)
                nc.tensor.matmul(
                    s_c[:, 512:1024],
                    khp_g[g][64:128, kc:kc + 128], qhh_t[64:128, qs],
                    start=True, stop=True, skip_group_check=True)

            def emit_exp(pr, j, s_c, t):
                p_t = p_hist[(t, pr)]
                pc = j * 1024
                if j == 0:   # ScalarE: exact exp, whole chunk
                    nc.scalar.activation(
                        p_t[:, pc:pc + 1024], s_c[:, 0:1024], EXP,
                        bias=bsh_t[:, 0:1], scale=1.0 / AEXP)
                else:        # VectorE: Schraudolph, whole chunk
                    nc.vector.tensor_scalar(
                        p_t[:, pc:pc + 1024].bitcast(i16),
                        s_c[:, 0:1024],
                        bdv_t[:, 0:1], 0.0, ADD, MAX)

            def emit_pv(pr, half, j, t):
                # 65-col V weight (V + ones column, no zero pad): smaller
                # LDWEIGHTS, output confined to partitions 0-64
                kt = 2 * pr + half
                g = pr // GPR
                lv = (kt - g * 2 * GPR) * 65
                p_t = p_hist[(t, pr)]
                pc = j * 1024 + half * 512
                nc.tensor.matmul(
                    o_accs[t][0:65, j * CHW:(j + 1) * CHW],
                    vpk_g[g][:, lv:lv + 65],
                    p_t[:, pc:pc + 512],
                    start=(pr == 0 and half == 0),
                    stop=(pr == NPAIR - 1 and half == 1),
                    skip_group_check=True)

            for k in range(NPER):
                sc = None
                if k < NPASS * NPAIR:
                    t, pr = k // NPAIR, k % NPAIR
                    if t == 0 and pr == 0:
                        # pass-0 accumulator + PE warmup before first scores
                        o_accs[0] = ps_o.tile([128, NPASS * CHW], f32,
                                              tag="o", name="o_acc")
                        for _ in range(NWARM):
                            nc.tensor.matmul(
                                o_accs[0][:, 0:512], wz[:, 0:128], wz,
                                start=True, stop=True,
                                skip_group_check=True)
                    sc = (t, pr)
                    p_hist[(t, pr)] = pp.tile([128, 2048], bf16,
                                              tag="p", name="p_t")
                pv = None
                if k >= TRAIL:
                    kp = k - TRAIL
                    pv = (kp // NPAIR, kp % NPAIR)
                    if pv[0] == 1 and pv[1] == 0:
                        # pass-1 accumulator: allocated after the pass-0
                        # ship copy so the pool WAR-snapshot includes it
                        o_accs[1] = ps_o.tile([128, NPASS * CHW], f32,
                                              tag="o", name="o_acc")

                # slots 0-1: scores chunks c0, c1 back to back (K weight
                # block loads once and both score slots use it while the
                # PV weights prefetch under the score streams)
                if sc:
                    s_c0 = ps_s.tile([128, 1024], f32, tag="s")
                    emit_scores(sc[1], 0, s_c0, sc[0])
                    emit_exp(sc[1], 0, s_c0, sc[0])
                    s_c1 = ps_s.tile([128, 1024], f32, tag="s")
                    emit_scores(sc[1], 1, s_c1, sc[0])
                    emit_exp(sc[1], 1, s_c1, sc[0])
                # slots 2-3: pvA
                if pv:
                    emit_pv(pv[1], 0, 0, pv[0])
                    emit_pv(pv[1], 0, 1, pv[0])
                elif sc and sc[0] == 0 and sc[1] < TRAIL:
                    for _ in range(2):
                        nc.tensor.matmul(
                            o_accs[0][:, 0:512], wz[:, 0:128], wz,
                            start=True, stop=True, skip_group_check=True)
                # slots 4-5: pvB
                if pv:
                    emit_pv(pv[1], 1, 0, pv[0])
                    if pv[1] == NPAIR - 1:
                        # ship chunk 0 on ScalarE right after its final
                        # PV (before pvB-c1), chunk 1 on VectorE after:
                        # the two copies run concurrently, halving the
                        # pass-transition / tail copy latency
                        t_ = pv[0]
                        oc = t_ * 1024
                        nc.scalar.activation(
                            o_sb[:, oc:oc + 512],
                            o_accs[t_][0:65, 0:512],
                            mybir.ActivationFunctionType.Copy)
                        nc.sync.dma_start(out=out[:, oc:oc + 512],
                                          in_=o_sb[:, oc:oc + 512])
                    kp_n = k + 1 - TRAIL
                    if kp_n < NPASS * NPAIR and prev_s_c1 is not None:
                        # tiny N=64 matmul carrying next period's vA
                        # weight: its implicit LDW sits one MM earlier in
                        # the queue, so the load runs under pvB-c0's
                        # stream and the real pvA-c0 LDW hits the
                        # same-AP fast path (scratch target: old score
                        # buffer, start=True-cleared before reuse)
                        pr_n = kp_n % NPAIR
                        g_ = pr_n // GPR
                        lv_ = (2 * pr_n - g_ * 2 * GPR) * 65
                        nc.tensor.matmul(
                            prev_s_c1[0:65, 0:64],
                            vpk_g[g_][:, lv_:lv_ + 65],
                            wz[:, 0:64],
                            start=True, stop=True, skip_group_check=True)
                    emit_pv(pv[1], 1, 1, pv[0])
                    if pv[1] == NPAIR - 1:
                        t_ = pv[0]
                        oc = t_ * 1024 + 512
                        nc.vector.tensor_copy(
                            o_sb[:, oc:oc + 512],
                            o_accs[t_][0:65, 512:1024])
                        nc.sync.dma_start(out=out[:, oc:oc + 512],
                                          in_=o_sb[:, oc:oc + 512])
                elif sc and sc[0] == 0 and sc[1] < TRAIL:
                    for _ in range(2):
                        nc.tensor.matmul(
                            o_accs[0][:, 0:512], wz[:, 0:128], wz,
                            start=True, stop=True, skip_group_check=True)
                if sc:
                    prev_s_c1 = s_c1
    nc.compile()
    return nc


def _prep_inputs(x):
    """Host-side shard + operand marshaling. Returns list of 8 in_maps."""
    import ml_dtypes
    bf16 = ml_dtypes.bfloat16
    t = np.ascontiguousarray(x, np.float32).reshape(B_, NTOK, C_)
    in_maps = []
    for b in range(B_):
        kv = t[b]                                   # [4096, 64]
        k_hi = kv.astype(bf16)
        kmax = float(np.linalg.norm(kv.astype(np.float64), axis=1).max())
        # packed even/odd K^T: rows 0-63 = tiles 0,2,..., rows 64-127 = 1,3,...
        kT = k_hi.T                                  # [64, 4096]
        ke = np.concatenate([kT[:, (2 * p) * 128:(2 * p + 1) * 128]
                             for p in range(NPAIR)], axis=1)
        ko = np.concatenate([kT[:, (2 * p + 1) * 128:(2 * p + 2) * 128]
                             for p in range(NPAIR)], axis=1)
        khp = np.concatenate([ke, ko])               # [128, 2048] bf16
        vpk = np.concatenate(
            [np.concatenate([kv[i * 128:(i + 1) * 128],
                             np.ones((128, 1), np.float32)], axis=1)
             for i in range(NKT)] + [np.zeros((128, 64), np.float32)],
            axis=1).astype(bf16)                    # [128, 32*65 + 64]
        for h in range(2):
            q = t[b, h * NQ:(h + 1) * NQ]           # [2048, 64]
            qa = (q.astype(bf16).astype(np.float32)
                  * np.float32(AEXP)).astype(bf16)
            qhh = np.concatenate([qa.T, qa.T])      # [128, 2048] bf16
            qn = np.linalg.norm(q.astype(np.float64), axis=1).max()
            shift = qn * kmax - MARGIN              # global per-core shift
            bsh = np.full((128, 1), -shift, np.float32)
            bdv = np.full((128, 1), 16256.0 - C_SCH - AEXP * shift,
                          np.float32)
            in_maps.append({
                "qhh": qhh, "khp": khp, "vpk": vpk, "bsh": bsh, "bdv": bdv,
            })
    return in_maps


def run(x, trace=False):
    from concourse.bass_utils import run_bass_kernel_spmd
    if "nc" not in _CACHE:
        _CACHE["nc"] = _build_nc()
    nc = _CACHE["nc"]
    in_maps = _prep_inputs(x)
    res = run_bass_kernel_spmd(
        nc, in_maps, core_ids=list(range(NCORES)), trace=trace,
    )
    full = np.empty((B_, NTOK, C_), np.float32)
    for b in range(B_):
        for h in range(2):
            o = res.results[2 * b + h]["out"]        # [65, 2048]
            full[b, h * NQ:(h + 1) * NQ] = (o[0:C_] / o[C_]).T
    return full.reshape(B_, D_, H_, W_, C_), res


def kernel(x):
    out, _ = run(x, trace=False)
    return out


# revision 24
# speedup vs baseline: 1.2197x; 1.2197x over previous
"""Trainium2 Bass kernel for nn_Denoiser_73598559584966.

Full-sequence self-attention (Q=K=V, no scaling) over x: [4, 16, 16, 16, 64]
  t = x.reshape(B, 4096, 64); out = softmax(t @ t^T) @ t

Sharding: 8 cores = 4 batches x 2 query-halves. Each core: 2048 queries
vs the full 4096 keys/values of its batch. No collectives.

v3: two passes over chunk-pairs, pair-inner, streamed as one pipeline.
Pass 0 covers query chunks 0,1; pass 1 covers chunks 2,3. Per pass the
output accumulator o_acc is only [128, 1024] f32 (2 PSUM banks), which
frees SIX banks for THREE [128, 1024] score buffers -- enough slack
that the exp engines never gate the PE's score-bank reuse (v2's 2-buffer
ping-pong made every half-period wait on a full exp op + 2 sem hops and
ran 2x slow).

Period (one pair p, 6 N=512 streams, ~1280ns):
  [ s(p,c0) | pvA(p-2,c0) pvA(p-2,c1) | s(p,c1) | pvB(p-2,c0) pvB(p-2,c1) ]
- PV trails the scores by TWO pairs so no PV ever waits on an exp op.
- Weight sequence K(p), vA(p-2), K(p) again (still resident -- the two
  score slots bracket only one other weight so K survives in its array
  slot), vB(p-2): 3 weight loads per 6 streams, each with >=1 full
  stream of prefetch lead (v1 paid ~210ns/3-streams of exposed loads).
- exp granularity is one whole chunk per op: ScalarE does exact exp on
  chunk c0 [128, 1024] (~1049ns), VectorE does Schraudolph bf16-bitcast
  exp on chunk c1 (~1263ns). Staggered score slots give each score
  buffer a ~1707ns reuse window vs ~1550/2400ns read-done chains.

K layout is packed even/odd: khp[0:64, p*128:+128] = K^T of tile 2p,
khp[64:128, same cols] = K^T of tile 2p+1 -> the pair's two concurrent
row-group score matmuls share one 128-col weight block (and K DMA
halves vs duplicating K^T). Softmax shift is global per core
(shift = max|q|*max|k| - 30); it cancels exactly in the ratio and
keeps relevant s-shift in bf16-normal / int16-safe range.

o_acc rows: 0-63 = V out, row 64 = denominator (ones column in the V
weights), 65+ = garbage from the 65-col-stride V weight blocks (never
read). Host divides and transposes while gathering shards.
"""
import numpy as np

B_, D_, H_, W_, C_ = 4, 16, 16, 16, 64
NTOK = D_ * H_ * W_          # 4096 tokens per batch
NQ = NTOK // 2               # 2048 queries per core
NCORES = 8
NKT = NTOK // 128            # 32 key tiles
NPAIR = NKT // 2             # 16 packed key-tile pairs
NPASS = 2                    # chunk-pair passes (chunks 2t, 2t+1)
CHW = 512                    # queries per chunk
NG = 4                       # DMA groups over key-tile pairs
GPR = NPAIR // NG            # 4 pairs per group

AEXP = 184.66350558899108    # 128 / ln 2  (bf16 Schraudolph scale)
C_SCH = 5.590103149414062    # Schraudolph bias-correction (bf16 grid)
MARGIN = 30.0                # shift = core score bound - MARGIN
NWARM = 0                    # real scores warm HAM themselves; warmups
                             # only delay the (DMA-gated) first score
TRAIL = 2                    # pv trails scores by TRAIL pairs

_CACHE = {}


def _build_nc():
    import concourse.bacc as bacc
    import concourse.mybir as mybir
    from concourse.tile import TileContext

    f32 = mybir.dt.float32
    i16 = mybir.dt.int16
    bf16 = mybir.dt.bfloat16
    EXP = mybir.ActivationFunctionType.Exp
    ADD = mybir.AluOpType.add
    MAX = mybir.AluOpType.max
    nc = bacc.Bacc("TRN2", target_bir_lowering=False, debug=False)

    qhh = nc.dram_tensor("qhh", [128, NQ], bf16, kind="ExternalInput")
    khp = nc.dram_tensor("khp", [128, NPAIR * 128], bf16, kind="ExternalInput")
    vpk = nc.dram_tensor("vpk", [128, NKT * 65 + 64], bf16,
                         kind="ExternalInput")
    bsh = nc.dram_tensor("bsh", [128, 1], f32, kind="ExternalInput")
    bdv = nc.dram_tensor("bdv", [128, 1], f32, kind="ExternalInput")
    out = nc.dram_tensor("out", [65, NQ], f32, kind="ExternalOutput")

    KGW = GPR * 128           # khp cols per DMA group (512)
    VGW = GPR * 2 * 65        # vpk cols per DMA group (520)
    NPER = NPASS * NPAIR + TRAIL
    with TileContext(nc) as tc:
        with (
            tc.tile_pool(name="const", bufs=1) as const,
            tc.tile_pool(name="pp", bufs=3) as pp,
            tc.tile_pool(name="sbo", bufs=1) as sbo,
            tc.tile_pool(name="ps_s", bufs=3, space="PSUM") as ps_s,
            tc.tile_pool(name="ps_o", bufs=1, space="PSUM") as ps_o,
        ):
            # ---- PE + ACT warmup during the DMA prefix ----
            wz = const.tile([128, 512], bf16, tag="wz")
            nc.vector.memset(wz, 0.0)
            wexp = const.tile([128, 1], f32, tag="wexp")
            nc.scalar.activation(wexp, wz[:, 0:1], EXP)  # pull exp table load

            # ---- input DMAs: first-needed operands first ----
            qhh_t = const.tile([128, NQ], bf16, tag="qhh")
            khp_g = []
            vpk_g = []
            for g in range(NG):
                kt_ = const.tile([128, KGW], bf16, tag=f"khp_{g}")
                khp_g.append(kt_)
                kt_ = const.tile([128, VGW + 63], bf16, tag=f"vpk_{g}")
                vpk_g.append(kt_)
            # qhh first: dma_start issues serialize ~0.5us apart on the
            # Sync engine and this 256KB piece is the first-score gate
            nc.sync.dma_start(out=qhh_t[:, 0:1024], in_=qhh[:, 0:1024])
            nc.sync.dma_start(out=khp_g[0][:, 0:128], in_=khp[:, 0:128])
            bsh_t = const.tile([128, 1], f32, tag="bsh")
            nc.sync.dma_start(out=bsh_t, in_=bsh[:, :])
            bdv_t = const.tile([128, 1], f32, tag="bdv")
            nc.sync.dma_start(out=bdv_t, in_=bdv[:, :])
            nc.sync.dma_start(out=vpk_g[0], in_=vpk[:, 0:VGW + 63])
            nc.sync.dma_start(out=khp_g[0][:, 128:KGW], in_=khp[:, 128:KGW])
            for g in range(1, NG):
                nc.sync.dma_start(
                    out=khp_g[g], in_=khp[:, g * KGW:(g + 1) * KGW])
                nc.sync.dma_start(
                    out=vpk_g[g], in_=vpk[:, g * VGW:(g + 1) * VGW + 63])
            nc.sync.dma_start(out=qhh_t[:, 1024:2048], in_=qhh[:, 1024:2048])

            o_accs = [None, None]        # per pass
            p_hist = {}                  # (pass, pair) -> p_t
            o_sb = sbo.tile([65, NQ], f32, tag="osb")

            def emit_scores(pr, j, s_c, t):
                g = pr // GPR
                kc = (pr - g * GPR) * 128
                c = 2 * t + j
                qs = slice(c * CHW, (c + 1) * CHW)
                nc.tensor.matmul(
                    s_c[:, 0:512],
                    khp_g[g][0:64, kc:kc + 128], qhh_t[0:64, qs],
                    start=True, stop=True, skip_group_check=True)
                nc.tensor.matmul(
                    s_c[:, 512:1024],
                    khp_g[g][64:128, kc:kc + 128], qhh_t[64:128, qs],
                    start=True, stop=True, skip_group_check=True)

            def emit_exp(pr, j, s_c, t):
                p_t = p_hist[(t, pr)]
                pc = j * 1024
                if j == 0:   # ScalarE: exact exp, whole chunk
                    nc.scalar.activation(
                        p_t[:, pc:pc + 1024], s_c[:, 0:1024], EXP,
                        bias=bsh_t[:, 0:1], scale=1.0 / AEXP)
                else:        # VectorE: Schraudolph, whole chunk
                    nc.vector.tensor_scalar(
                        p_t[:, pc:pc + 1024].bitcast(i16),
                        s_c[:, 0:1024],
                        bdv_t[:, 0:1], 0.0, ADD, MAX)

            def emit_pv(pr, half, j, t):
                # 65-col V weight (V + ones column, no zero pad): smaller
                # LDWEIGHTS, output confined to partitions 0-64
                kt = 2 * pr + half
                g = pr // GPR
                lv = (kt - g * 2 * GPR) * 65
                p_t = p_hist[(t, pr)]
                pc = j * 1024 + half * 512
                nc.tensor.matmul(
                    o_accs[t][0:65, j * CHW:(j + 1) * CHW],
                    vpk_g[g][:, lv:lv + 65],
                    p_t[:, pc:pc + 512],
                    start=(pr == 0 and half == 0),
                    stop=(pr == NPAIR - 1 and half == 1),
                    skip_group_check=True)

            for k in range(NPER):
                sc = None
                if k < NPASS * NPAIR:
                    t, pr = k // NPAIR, k % NPAIR
                    if t == 0 and pr == 0:
                        # pass-0 accumulator + PE warmup before first scores
                        o_accs[0] = ps_o.tile([128, NPASS * CHW], f32,
                                              tag="o", name="o_acc")
                        for _ in range(NWARM):
                            nc.tensor.matmul(
                                o_accs[0][:, 0:512], wz[:, 0:128], wz,
                                start=True, stop=True,
                                skip_group_check=True)
                    sc = (t, pr)
                    p_hist[(t, pr)] = pp.tile([128, 2048], bf16,
                                              tag="p", name="p_t")
                pv = None
                if k >= TRAIL:
                    kp = k - TRAIL
                    pv = (kp // NPAIR, kp % NPAIR)
                    if pv[0] == 1 and pv[1] == 0:
                        # pass-1 accumulator: allocated after the pass-0
                        # ship copy so the pool WAR-snapshot includes it
                        o_accs[1] = ps_o.tile([128, NPASS * CHW], f32,
                                              tag="o", name="o_acc")

                # slots 0-1: scores chunks c0, c1 back to back (K weight
                # block loads once and both score slots use it while the
                # PV weights prefetch under the score streams)
                if sc:
                    s_c0 = ps_s.tile([128, 1024], f32, tag="s")
                    emit_scores(sc[1], 0, s_c0, sc[0])
                    emit_exp(sc[1], 0, s_c0, sc[0])
                    s_c1 = ps_s.tile([128, 1024], f32, tag="s")
                    emit_scores(sc[1], 1, s_c1, sc[0])
                    emit_exp(sc[1], 1, s_c1, sc[0])
                # slots 2-3: pvA
                if pv:
                    emit_pv(pv[1], 0, 0, pv[0])
                    emit_pv(pv[1], 0, 1, pv[0])
                elif sc and sc[0] == 0 and sc[1] < TRAIL:
                    for _ in range(2):
                        nc.tensor.matmul(
                            o_accs[0][:, 0:512], wz[:, 0:128], wz,
                            start=True, stop=True, skip_group_check=True)
                # slots 4-5: pvB
                if pv:
                    emit_pv(pv[1], 1, 0, pv[0])
                    if pv[1] == NPAIR - 1:
                        # ship chunk 0 on ScalarE right after its final
                        # PV (before pvB-c1), chunk 1 on VectorE after:
                        # the two copies run concurrently, halving the
                        # pass-transition / tail copy latency
                        t_ = pv[0]
                        oc = t_ * 1024
                        nc.scalar.activation(
                            o_sb[:, oc:oc + 512],
                            o_accs[t_][0:65, 0:512],
                            mybir.ActivationFunctionType.Copy)
                        nc.sync.dma_start(out=out[:, oc:oc + 512],
                                          in_=o_sb[:, oc:oc + 512])
                    emit_pv(pv[1], 1, 1, pv[0])
                    if pv[1] == NPAIR - 1:
                        t_ = pv[0]
                        oc = t_ * 1024 + 512
                        nc.vector.tensor_copy(
                            o_sb[:, oc:oc + 512],
                            o_accs[t_][0:65, 512:1024])
                        nc.sync.dma_start(out=out[:, oc:oc + 512],
                                          in_=o_sb[:, oc:oc + 512])
                elif sc and sc[0] == 0 and sc[1] < TRAIL:
                    for _ in range(2):
                        nc.tensor.matmul(
                            o_accs[0][:, 0:512], wz[:, 0:128], wz,
                            start=True, stop=True, skip_group_check=True)
    nc.compile()
    return nc


def _prep_inputs(x):
    """Host-side shard + operand marshaling. Returns list of 8 in_maps."""
    import ml_dtypes
    bf16 = ml_dtypes.bfloat16
    t = np.ascontiguousarray(x, np.float32).reshape(B_, NTOK, C_)
    in_maps = []
    for b in range(B_):
        kv = t[b]                                   # [4096, 64]
        k_hi = kv.astype(bf16)
        kmax = float(np.linalg.norm(kv.astype(np.float64), axis=1).max())
        # packed even/odd K^T: rows 0-63 = tiles 0,2,..., rows 64-127 = 1,3,...
        kT = k_hi.T                                  # [64, 4096]
        ke = np.concatenate([kT[:, (2 * p) * 128:(2 * p + 1) * 128]
                             for p in range(NPAIR)], axis=1)
        ko = np.concatenate([kT[:, (2 * p + 1) * 128:(2 * p + 2) * 128]
                             for p in range(NPAIR)], axis=1)
        khp = np.concatenate([ke, ko])               # [128, 2048] bf16
        vpk = np.concatenate(
            [np.concatenate([kv[i * 128:(i + 1) * 128],
                             np.ones((128, 1), np.float32)], axis=1)
             for i in range(NKT)] + [np.zeros((128, 64), np.float32)],
            axis=1).astype(bf16)                    # [128, 32*65 + 64]
        for h in range(2):
            q = t[b, h * NQ:(h + 1) * NQ]           # [2048, 64]
            qa = (q.astype(bf16).astype(np.float32)
                  * np.float32(AEXP)).astype(bf16)
            qhh = np.concatenate([qa.T, qa.T])      # [128, 2048] bf16
            qn = np.linalg.norm(q.astype(np.float64), axis=1).max()
            shift = qn * kmax - MARGIN              # global per-core shift
            bsh = np.full((128, 1), -shift, np.float32)
            bdv = np.full((128, 1), 16256.0 - C_SCH - AEXP * shift,
                          np.float32)
            in_maps.append({
                "qhh": qhh, "khp": khp, "vpk": vpk, "bsh": bsh, "bdv": bdv,
            })
    return in_maps


def run(x, trace=False):
    from concourse.bass_utils import run_bass_kernel_spmd
    if "nc" not in _CACHE:
        _CACHE["nc"] = _build_nc()
    nc = _CACHE["nc"]
    in_maps = _prep_inputs(x)
    res = run_bass_kernel_spmd(
        nc, in_maps, core_ids=list(range(NCORES)), trace=trace,
    )
    full = np.empty((B_, NTOK, C_), np.float32)
    for b in range(B_):
        for h in range(2):
            o = res.results[2 * b + h]["out"]        # [65, 2048]
            full[b, h * NQ:(h + 1) * NQ] = (o[0:C_] / o[C_]).T
    return full.reshape(B_, D_, H_, W_, C_), res


def kernel(x):
    out, _ = run(x, trace=False)
    return out
